# revision 51
# baseline (speedup 1.0000x reference)
"""VQ codebook (nn_CodeBook) Trainium2 Bass kernel (raw Bass, manual sync).

Shapes (hardcoded): z [8192, 1024] f32, W [32, 1024, 32] f32
(S=32 codebooks of K=1024 codes, D=32).

Forward math: per split s, row n: k* = argmin_k ||z_ns - W_sk||^2; output is
numerically zq = W[s, k*] (straight-through), and
loss = S*(1+beta)*mean((zq - zc)^2) = S*(1+beta)*mean over (n,s) of d2_min.

Distribution: 8 cores data-parallel over rows; W replicated; per-core partial
results combined on host (the unshard step).

Device pipeline per [128 rows x 1024 codes] tile (256 tiles/core):
  - PE: scores s'[n,k] = 2^30*(||w_k||^2 - 2 z_n.w_k): two matmuls with
    contract dim 32 (z^T and -2^31*W^T staged with four splits packed
    vertically per 128 partitions) plus two rank-1 matmuls accumulating the
    2^30*||w||^2 bias row. The 2^30 power-of-two pre-scale is bitwise-lossless
    so the argmin is unchanged.
  - DVE: exact row-min via tensor_reduce straight from PSUM, then
    MaxIndex(scores, min x8) -> FIRST index equal to the min, which matches
    the reference argmin tie semantics exactly.
Synchronization is manual (one semaphore wait per instruction, 3-deep PSUM
rotation PE<->DVE). Host finishes: gathers zq = W[s, idx] and computes the
loss from sum(z^2) + sum of the mins.
"""

import numpy as np

S, K, D = 32, 1024, 32
N, IDIM = 8192, 1024
NCORES = 8
NSH = N // NCORES  # rows per core = 1024
NCH = NSH // 128  # n-chunks per core = 8
NTILE = S * NCH  # score tiles per core = 256
NG = S // 4  # split quad-groups = 8
NPS = 3  # PSUM rotation depth
BETA = 0.001
SCALE = float(2**30)
FREE = NSH + 3 * K + 128  # z | w | b | iota | ones per group

_CACHE = {}


def _build_bass():
    from contextlib import ExitStack

    import concourse.bass as bass
    import concourse.mybir as mb

    f32, u32 = mb.dt.float32, mb.dt.uint32

    nc = bass.Bass()
    inp_d = nc.dram_tensor("inp", [128, NG, FREE], f32, kind="ExternalInput")
    out_d = nc.dram_tensor("outp", [128, NTILE * 9], u32, kind="ExternalOutput")

    with ExitStack() as ctx:
        stage = ctx.enter_context(nc.sbuf_tensor([128, NG, FREE], f32))
        res = ctx.enter_context(nc.sbuf_tensor([128, NTILE * 9], u32))
        sb0 = ctx.enter_context(nc.sbuf_tensor([128, K], f32))
        sb1 = ctx.enter_context(nc.sbuf_tensor([128, K], f32))
        sb2 = ctx.enter_context(nc.sbuf_tensor([128, K], f32))
        ssbs = [sb0, sb1, sb2]
        ps0 = ctx.enter_context(nc.psum_tensor([128, K], f32))
        ps1 = ctx.enter_context(nc.psum_tensor([128, K], f32))
        ps2 = ctx.enter_context(nc.psum_tensor([128, K], f32))
        pss = [ps0, ps1, ps2]
        dma_sem = ctx.enter_context(nc.semaphore())
        pe_sem = ctx.enter_context(nc.semaphore())
        act_sem = ctx.enter_context(nc.semaphore())
        dve_sem = ctx.enter_context(nc.semaphore())
        block = ctx.enter_context(nc.Block())

        @block.sync
        def _(sync):
            sync.dma_start(stage[:], inp_d[:]).then_inc(dma_sem, 16)
            # final store: wait for the last DVE tile
            sync.wait_ge(dve_sem, NTILE)
            sync.dma_start(out_d[:], res[:]).then_inc(dma_sem, 16)

        @block.tensor
        def _(tensor):
            t = 0
            for s in range(S):
                g, q = s // 4, (s % 4) * 32
                wof, bof, oof = NSH, NSH + K, NSH + 3 * K
                for c in range(NCH):
                    ps = pss[t % NPS]
                    if t == 0:
                        tensor.wait_ge(dma_sem, 16)
                    elif t >= NPS:
                        # PSUM slot reuse: ACT copied tile t-NPS out already
                        tensor.wait_ge(act_sem, t - NPS + 1)
                    lhsT = stage[q : q + 32, g, c * 128 : (c + 1) * 128]
                    nc.tensor.matmul(ps[:, 0:512], lhsT,
                                     stage[q : q + 32, g, wof : wof + 512],
                                     start=True, stop=False,
                                     skip_group_check=True, tile_position=(q, 0))
                    nc.tensor.matmul(ps[:, 512:1024], lhsT,
                                     stage[q : q + 32, g, wof + 512 : wof + 1024],
                                     start=True, stop=False,
                                     skip_group_check=True, tile_position=(q, 0))
                    nc.tensor.matmul(ps[:, 0:512],
                                     stage[q : q + 1, g, oof : oof + 128],
                                     stage[q : q + 1, g, bof : bof + 512],
                                     start=False, stop=True,
                                     skip_group_check=True, tile_position=(q, 0))
                    nc.tensor.matmul(ps[:, 512:1024],
                                     stage[q : q + 1, g, oof : oof + 128],
                                     stage[q : q + 1, g, bof + 512 : bof + 1024],
                                     start=False, stop=True,
                                     skip_group_check=True,
                                     tile_position=(q, 0)).then_inc(pe_sem, 1)
                    t += 1

        @block.scalar
        def _(scalar):
            for t in range(NTILE):
                ps = pss[t % NPS]
                ssb = ssbs[t % NPS]
                scalar.wait_ge(pe_sem, t + 1) if t < NPS else scalar.wait_ge(pe_sem, t + 1)
                if t >= NPS:
                    # SBUF slot reuse: DVE must be done with tile t-NPS
                    scalar.wait_ge(dve_sem, t - NPS + 1)
                nc.scalar.copy(ssb[:], ps[:]).then_inc(act_sem, 1)

        @block.vector
        def _(vector):
            iof = NSH + 2 * K
            iota = stage[:, 0, iof : iof + K]
            for t in range(NTILE):
                ssb = ssbs[t % NPS]
                vector.wait_ge(act_sem, t + 1)
                m8 = res[:, NTILE + t * 8 : NTILE + t * 8 + 8].bitcast(f32)
                nc.vector.max(m8, ssb[:])
                # sum over k of (score==max)*(1024-k) -> 1024-k* (host inverts;
                # bitwise ties detected on host via m8[0]==m8[1] and recomputed)
                nc.vector.scalar_tensor_tensor(
                    out=ssb[:],
                    in0=ssb[:],
                    scalar=m8[:, 0:1],
                    in1=iota,
                    op0=mb.AluOpType.is_equal,
                    op1=mb.AluOpType.mult,
                    accum_out=res[:, t : t + 1].bitcast(f32),
                ).then_inc(dve_sem, 1)

    return nc


def _prep_inputs(z, W):
    """Host-side sharding/staging. Returns per-core input maps."""
    z = np.ascontiguousarray(z, dtype=np.float32)
    W = np.ascontiguousarray(W, dtype=np.float32)

    b = (W**2).sum(axis=2, dtype=np.float32)  # [S, K]
    base = np.zeros((128, NG, FREE), dtype=np.float32)
    base[:, :, NSH + 3 * K :] = 1.0  # ones block for the rank-1 bias matmuls
    base[:, :, NSH + 2 * K : NSH + 3 * K] = (
        float(K) - np.arange(K, dtype=np.float32)[None, None, :]
    )
    for s in range(S):
        g, q = s // 4, (s % 4) * 32
        base[q : q + 32, g, NSH : NSH + K] = (2.0 * SCALE) * W[s].T  # [d, k]
        base[q, g, NSH + K : NSH + 2 * K] = -SCALE * b[s]

    in_maps = []
    for core in range(NCORES):
        zs = z[core * NSH : (core + 1) * NSH]  # [NSH, 1024]
        inp = base.copy()
        zc = zs.reshape(NSH, S, D).transpose(1, 2, 0)  # [s, d, n]
        for s in range(S):
            g, q = s // 4, (s % 4) * 32
            inp[q : q + 32, g, 0:NSH] = zc[s]
        in_maps.append({"inp": inp})
    return in_maps


def _postprocess(results, z, W):
    zq = np.empty((N, IDIM), dtype=np.float32)
    sse = float(np.square(z, dtype=np.float64).sum())
    for core, r in enumerate(results):
        outp = r["outp"]
        # cols [0:NTILE): sum-encoded 1024-k*; after: top-8 negated mins
        ksum = outp[:, :NTILE].view(np.float32)
        top8 = outp[:, NTILE:].view(np.float32).reshape(128, NTILE, 8)
        mall = -top8[:, :, 0]
        idx = (float(K) - ksum).astype(np.int64)  # [p, t]
        tie = top8[:, :, 0] == top8[:, :, 1]
        bad = tie | (idx < 0) | (idx >= K)
        if bad.any():
            pb, tb = np.nonzero(bad)
            for p, t in zip(pb, tb):
                s, c = divmod(int(t), NCH)
                n = core * NSH + c * 128 + int(p)
                zrow = z[n, s * D : (s + 1) * D]
                d2 = ((W[s] - zrow[None, :]) ** 2).sum(axis=1)
                idx[p, t] = int(np.argmin(d2))
        idx = idx.reshape(128, S, NCH)
        zq_c = W[np.arange(S)[None, :, None], idx]  # [p, s, c, d]
        zq[core * NSH : (core + 1) * NSH] = (
            zq_c.transpose(2, 0, 1, 3).reshape(NSH, IDIM)
        )
        sse += float(mall.astype(np.float64).sum()) / float(SCALE)
    loss = np.float32(S * (1.0 + BETA) * sse / (N * IDIM))
    return zq, loss


def kernel(z, W):
    from concourse.bass_utils import run_bass_kernel_spmd

    if "nc" not in _CACHE:
        _CACHE["nc"] = _build_bass()
    nc = _CACHE["nc"]

    z = np.ascontiguousarray(z, dtype=np.float32)
    W = np.ascontiguousarray(W, dtype=np.float32)
    in_maps = _prep_inputs(z, W)
    res = run_bass_kernel_spmd(nc, in_maps, core_ids=list(range(NCORES)))
    return _postprocess(res.results, z, W)


if __name__ == "__main__":
    rng = np.random.default_rng(0)
    z = rng.standard_normal((N, IDIM), dtype=np.float32)
    W = rng.standard_normal((S, K, D), dtype=np.float32)
    zq, loss = kernel(z, W)
    print(zq.shape, loss)
